# revision 31
# baseline (speedup 1.0000x reference)
"""CRF loss (forward-algorithm partition function) on 8 Trainium2 cores.

Strategy (segment-parallel matrix chain)
----------------------------------------
Batch (B=64) is sharded 8 ways -> 8 sequences per core.  The log-space scan
is computed in *linear* space:  with  E_l = exp(scores_l - C),
C = log(T) + 0.5, the recurrence becomes  w_l = E_l^T w_{l-1}.

Instead of a 511-step sequential vector chain (latency-bound: each tiny
matvec waits on the previous step's PSUM->SBUF copy; 388-605us), each chain
is split into S=8 *segments* of 64 matrices (one identity pad at the global
front).  Each segment reduces independently via matrix-matrix products
A_j = E_j^T A_{j-1}  (A_0 = I), giving 64 independent streams per core
-> the TensorE pipeline stays full and no step waits on any other stream.
The host combines the 8 segment matrices per chain in float64 (trivial
flops) and applies gold-path energy / softmax weighting.

Pipeline structure (per round, 64 rounds; measured ~199us, PE+DVE >98%
busy in steady state):
  PE:  64 independent [64x64]@[64x64] bf16 matmuls (streams packed 2/tile,
       operands at partition offsets 0/64 -- verified in CoreSim)
  DVE: 4 copies [128,512] PSUM->SBUF bf16, one per group-pair of 8 streams.
       The 4-way group-pair rotation hides each copy's latency behind the
       other three group-pairs' matmuls (going from 2-way to 4-way took HW
       from 234us to 199us).  Each PSUM parity tile is exactly one 2KB bank
       so PE-writes and DVE-reads of adjacent rounds never share a bank;
       the 8 tiles fill PSUM exactly.
  ACT: pure exp(x - C) stream on [128, 1024] tiles (a 16-step block is
       exp'd once and consumed over 16 rounds -> off the round path)
  DMA: host pre-packs the score image in bf16 (the matmul consumes bf16
       E-matrices regardless; exp(bf16(s)-C) vs bf16(exp(s-C)) is the same
       information loss) with every descriptor 2KB contiguous per partition
       (<512B descriptors pay a 2x RMW penalty).  32MB/core at ~330GB/s
       (~100us) leaves the kernel compute-paced: the fp32 wire measured
       295us with DMA 98% busy over the whole span.
"""

import os
import threading
import numpy as np
import ml_dtypes

L, B, T = 512, 64, 64
NCORES = 8
B_LOC = B // NCORES            # 8 sequences per core
NSEG = 8                       # segments per chain
NSTEP = 64                     # matrices per segment (incl. 1 identity pad)
NPAIR = 32                     # stream pairs per core: q = s*4 + a
NGP = NSEG // 2                # group-pairs (copy/PSUM granularity: 8 pairs)
W_LIST = [16, 16, 16, 16]      # steps per DMA/exp block (sum = NSTEP)
NBLK = len(W_LIST)
W_OFF = [sum(W_LIST[:k]) for k in range(NBLK)]
C_SHIFT = float(np.log(T) + 0.5)
START_TAG = 0
END_TAG = 1
NEG = -1e30                    # "minus infinity" for identity-pad off-diagonals
# bf16 rounding of the pad diagonal: the device applies exp(bf16(C) - C) != 1
# on each pad/masked step; the host subtracts this known constant exactly.
C_BF = float(np.asarray(C_SHIFT, dtype=ml_dtypes.bfloat16))

_nc_cache = [None]
_nc_lock = threading.Lock()
LAST_RESULTS = [None]          # test.py reads exec_time_ns from here


def _build_nc():
    import concourse.bacc as bacc
    import concourse.mybir as mybir
    import concourse.tile as tile

    dt = mybir.dt
    nc = bacc.Bacc("TRN2", target_bir_lowering=False, debug=False)

    # [pair, partition, (step, u)] bf16 -- 16KB contiguous per partition;
    # each block DMA slices W_k*128B >= 2KB per partition.
    img_d = nc.declare_dram_parameter(
        "img", [NPAIR, 128, NSTEP * T], dt.bfloat16, isOutput=False
    )
    eye_d = nc.declare_dram_parameter("eye2", [128, T], dt.float32, isOutput=False)
    out_d = nc.declare_dram_parameter(
        "m_out", [128, NSEG * 4 * T], dt.float32, isOutput=True
    )

    with tile.TileContext(nc) as tc:
        with (
            tc.tile_pool(name="raw", bufs=8) as raw_pool,
            tc.tile_pool(name="exp", bufs=2) as exp_pool,
            tc.tile_pool(name="state", bufs=1) as state_pool,
            tc.tile_pool(name="psum", bufs=1, space="PSUM") as psum_pool,
        ):
            eye_stage = state_pool.tile([128, T], dt.float32)
            eye_bf = state_pool.tile([128, T], dt.bfloat16)
            bias_c = state_pool.tile([128, 1], dt.float32)
            nc.gpsimd.memset(bias_c[:], -C_SHIFT)
            out_stage = state_pool.tile([128, NSEG * 4 * T], dt.float32)
            # accumulators per (group-pair gp in {0,1}, parity): group g =
            # 2*gp + gi at cols gi*256 + a*64; chain 2a+h at partitions h*64.
            acc = [
                [
                    state_pool.tile([128, 8 * T], dt.bfloat16, name=f"acc_gp{gp}p{p}")
                    for p in range(2)
                ]
                for gp in range(NGP)
            ]
            # PSUM: one [128, 512] tile (= exactly one 2KB bank) per
            # (group-pair, parity); parity-0 tiles first so the two parities
            # of a group-pair land in different banks (PE-W vs DVE-R on the
            # same bank serialize in hardware).
            psum_tiles = {}
            for p in range(2):
                for gp in range(NGP):
                    psum_tiles[(gp, p)] = psum_pool.tile(
                        [128, 8 * T], dt.float32, name=f"ps_gp{gp}p{p}"
                    )

            nc.sync.dma_start(eye_stage[:], eye_d[:])
            nc.vector.tensor_copy(eye_bf[:], eye_stage[:])
            for gp in range(NGP):
                for col in range(8):
                    nc.vector.tensor_copy(
                        acc[gp][0][:, col * T : (col + 1) * T], eye_bf[:]
                    )

            # ---- stream in all score blocks (SP queue drains as raw bufs
            # free up; order (k, q) matches consumption order).  One exp
            # instruction per (pair, block): batching two pairs per exp
            # measured slower on HW (207us vs 199us) -- coarser exp
            # granularity delays the first consumer rounds. ----
            raw_tiles = {}
            for k in range(NBLK):
                for q in range(NPAIR):
                    t_raw = raw_pool.tile([128, W_LIST[k] * T], dt.bfloat16, tag="raw")
                    nc.sync.dma_start(
                        t_raw[:],
                        img_d[q, :, W_OFF[k] * T : (W_OFF[k] + W_LIST[k]) * T],
                    )
                    raw_tiles[(q, k)] = t_raw

            # All PSUM->SBUF copies stay on DVE: offloading some to ScalarE
            # measured SLOWER on hardware (238-262us vs 234us) -- switching
            # the ACT function between Exp and Copy reloads the activation
            # table (~1.3us per switch) and ACT's PSUM port is slower.
            exp_tiles = {}

            def emit_exp(q, k):
                t = exp_pool.tile([128, W_LIST[k] * T], dt.bfloat16, tag=f"exp{q}")
                nc.scalar.activation(
                    t[:],
                    raw_tiles[(q, k)][:],
                    mybir.ActivationFunctionType.Exp,
                    bias=bias_c[:, 0:1],
                )
                exp_tiles[(q, k)] = t

            for q in range(NPAIR):
                emit_exp(q, 0)

            # ---- staggered-wavefront rounds over 64 independent streams.
            # Group-pair g starts DSTAG[g] waves late, matching when ITS
            # block-0 exps land (gp0 ~11us, gp3 ~34us): the engines do
            # gp0-gp2's early rounds during the window that round-major
            # lockstep would leave idle waiting for gp3's exps. ----
            DSTAG = [0, 4, 8, 12]
            for t in range(NSTEP + DSTAG[-1]):
                for gp in range(NGP):
                    r = t - DSTAG[gp]
                    if not (0 <= r < NSTEP):
                        continue
                    kblk = max(k for k in range(NBLK) if W_OFF[k] <= r)
                    w = r - W_OFF[kblk]
                    rp = r % 2
                    last = r == NSTEP - 1
                    ps = psum_tiles[(gp, rp)]
                    a_in = acc[gp][rp]
                    for gi in range(2):
                        g = 2 * gp + gi
                        for a in range(4):
                            q = g * 4 + a
                            et = exp_tiles[(q, kblk)]
                            lo = slice(0, T)
                            hi = slice(T, 2 * T)
                            cw = slice(w * T, (w + 1) * T)
                            ca = slice(gi * 4 * T + a * T, gi * 4 * T + (a + 1) * T)
                            nc.tensor.matmul(
                                ps[lo, ca], et[lo, cw], a_in[lo, ca],
                                start=True, stop=True,
                            )
                            nc.tensor.matmul(
                                ps[hi, ca], et[hi, cw], a_in[hi, ca],
                                start=True, stop=True,
                            )
                    # one [128,512] DVE copy per group-pair: bf16 for the
                    # next round, fp32 to the staging tile on the last one.
                    # (Offloading copies to ScalarE measured slower on HW
                    # every time it was tried: 201/238/262us vs 199us.)
                    if last:
                        dst = out_stage[:, gp * 8 * T : (gp + 1) * 8 * T]
                    else:
                        dst = acc[gp][(r + 1) % 2][:]
                    nc.vector.tensor_copy(dst, ps[:])
                # spread the next block's 32 exps over gp0's current block
                # rounds (ascending q = ascending gp, so each exp's buffer
                # WAR clears just before emission under the stagger)
                if t < NSTEP:
                    kb0 = max(k for k in range(NBLK) if W_OFF[k] <= t)
                    if kb0 < NBLK - 1:
                        w0 = t - W_OFF[kb0]
                        wk = W_LIST[kb0]
                        for qn in range(w0 * NPAIR // wk, (w0 + 1) * NPAIR // wk):
                            emit_exp(qn, kb0 + 1)

            nc.sync.dma_start(out_d[:], out_stage[:])
    nc.compile()
    return nc


def _get_nc():
    with _nc_lock:
        if _nc_cache[0] is None:
            _nc_cache[0] = _build_nc()
        return _nc_cache[0]


def _ensure_axon_hooks():
    """Provide antenv.axon_hooks (missing in this image) so that
    run_bass_kernel_spmd(trace=True) can register the NTFF profile hook."""
    import sys
    import types

    try:
        import antenv.axon_hooks  # noqa: F401
        return
    except ImportError:
        pass
    import antenv

    mod = types.ModuleType("antenv.axon_hooks")
    _hook = [None]
    mod.set_axon_ntff_profile_hook = lambda h: _hook.__setitem__(0, h)
    mod.get_axon_ntff_profile_hook = lambda: _hook[0]
    sys.modules["antenv.axon_hooks"] = mod
    antenv.axon_hooks = mod
    try:
        from trn_agent_boot.trn_boot import _ntff_profile_via_ctypes

        h = _ntff_profile_via_ctypes("/opt/axon/libaxon_pjrt.so")
        if h is not None:
            mod.set_axon_ntff_profile_hook(h)
    except Exception:
        pass


def _build_image(scores, mask, mask_all, c):
    """Per-core DMA image [NPAIR, 128, NSTEP*T] bf16.

    img[(s,a), (h,t), (j,u)] = padded[s*128 + j, (a,h), t, u]
    where padded[0] is the identity-pad matrix (diag +C, off-diag -1e30 so
    exp(x - C) ~= I) and padded[m] = scores[m] for m >= 1.
    """
    sh = scores[:, c * B_LOC : (c + 1) * B_LOC]  # (512, 8, 64, 64) view
    padded = np.empty((L, B_LOC, T, T), dtype=np.float32)
    padded[1:] = sh[1:]
    pad = np.full((T, T), NEG, dtype=np.float32)
    np.fill_diagonal(pad, C_SHIFT)
    padded[0] = pad
    if not mask_all:
        # a masked step must leave the partition unchanged: exp(x - C) ~= I
        mloc = mask[:, c * B_LOC : (c + 1) * B_LOC]
        ls, lb = np.nonzero(~mloc)
        for li, bi in zip(ls, lb):
            if li >= 1:
                padded[li, bi] = pad
    padded = padded.astype(ml_dtypes.bfloat16)
    # (s, a, h, t, j, u) <- (m=(s,j), c=(a,h), t, u)
    v = padded.reshape(NSEG, NSTEP, 4, 2, T, T)
    img = np.ascontiguousarray(v.transpose(0, 2, 3, 4, 1, 5)).reshape(
        NPAIR, 128, NSTEP * T
    )
    return img


def kernel(scores, target, mask, antor_score, aid, **_unused):
    from concourse.bass_utils import run_bass_kernel_spmd

    scores = np.asarray(scores, dtype=np.float32)
    target = np.asarray(target)
    mask = np.asarray(mask)
    antor_score = np.asarray(antor_score, dtype=np.float32)
    aid = int(np.asarray(aid))
    assert scores.shape == (L, B, T, T), scores.shape

    mask_all = bool(mask.all())

    # ---- host prep: initial vectors + per-core DMA images ----
    p0 = scores[0, :, START_TAG, :].astype(np.float64)          # (B, T)
    s0 = p0.max(axis=1)                                          # (B,)
    w0 = np.exp(p0 - s0[:, None])                                # (B, T) f64

    eye2 = np.tile(np.eye(T, dtype=np.float32), (2, 1))          # (128, 64)

    imgs = [None] * NCORES
    threads = [
        threading.Thread(
            target=lambda c=c: imgs.__setitem__(
                c, _build_image(scores, mask, mask_all, c)
            )
        )
        for c in range(NCORES)
    ]
    for t in threads:
        t.start()
    for t in threads:
        t.join()

    in_maps = [{"img": imgs[c], "eye2": eye2} for c in range(NCORES)]

    nc = _get_nc()
    do_trace = bool(int(os.environ.get("KERNEL_TRACE", "0")))
    if do_trace:
        _ensure_axon_hooks()
    try:
        res = run_bass_kernel_spmd(nc, in_maps, list(range(NCORES)), trace=do_trace)
    except Exception:
        if not do_trace:
            raise
        res = run_bass_kernel_spmd(nc, in_maps, list(range(NCORES)), trace=False)
    LAST_RESULTS[0] = res

    # ---- host combine (float64): Z_b = log(w[END]) + renorms + 511*C + s0 ----
    # m_out[(h,t'), gp*512 + gi*256 + a*64 + n] = M_{chain 2a+h, seg 2gp+gi}
    Z = 0.0
    for c in range(NCORES):
        out = np.asarray(res.results[c]["m_out"], dtype=np.float64)
        for bl in range(B_LOC):
            a, h = bl // 2, bl % 2
            b = c * B_LOC + bl
            w = w0[b].copy()
            logacc = 0.0
            for s in range(NSEG):
                col = s * 4 * T + a * T
                M = out[h * T : (h + 1) * T, col : col + T]
                w = M @ w
                mx = w.max()
                w /= mx
                logacc += np.log(mx)
            # each pad/masked step applied exp(C_BF - C) instead of exactly 1
            npad = 1 if mask_all else 1 + int((~mask[1:, b]).sum())
            Z += (
                np.log(w[END_TAG]) + logacc + s0[b]
                + (L - 1) * C_SHIFT - npad * (C_BF - C_SHIFT)
            )

    maskf = mask.astype(np.float64)
    tg = np.take_along_axis(
        scores.reshape(L, B, T * T), np.asarray(target, np.int64)[:, :, None], axis=2
    )[..., 0]
    tg_energy = float((tg * maskf).sum())

    a = antor_score.astype(np.float64)
    wsm = np.exp(a - a.max())
    wsm /= wsm.sum()
    loss = (Z - tg_energy) * wsm[aid] / B
    return np.float32(loss)


# revision 32
# speedup vs baseline: 1.0847x; 1.0847x over previous
"""CRF loss (forward-algorithm partition function) on 8 Trainium2 cores.

Strategy (segment-parallel matrix chain)
----------------------------------------
Batch (B=64) is sharded 8 ways -> 8 sequences per core.  The log-space scan
is computed in *linear* space:  with  E_l = exp(scores_l - C),
C = log(T) + 0.5, the recurrence becomes  w_l = E_l^T w_{l-1}.

Instead of a 511-step sequential vector chain (latency-bound: each tiny
matvec waits on the previous step's PSUM->SBUF copy; 388-605us), each chain
is split into S=8 *segments* of 64 matrices (one identity pad at the global
front).  Each segment reduces independently via matrix-matrix products
A_j = E_j^T A_{j-1}  (A_0 = I), giving 64 independent streams per core
-> the TensorE pipeline stays full and no step waits on any other stream.
The host combines the 8 segment matrices per chain in float64 (trivial
flops) and applies gold-path energy / softmax weighting.

Pipeline structure (per round, 64 rounds; measured ~199us, PE+DVE >98%
busy in steady state):
  PE:  64 independent [64x64]@[64x64] bf16 matmuls (streams packed 2/tile,
       operands at partition offsets 0/64 -- verified in CoreSim)
  DVE: 4 copies [128,512] PSUM->SBUF bf16, one per group-pair of 8 streams.
       The 4-way group-pair rotation hides each copy's latency behind the
       other three group-pairs' matmuls (going from 2-way to 4-way took HW
       from 234us to 199us).  Each PSUM parity tile is exactly one 2KB bank
       so PE-writes and DVE-reads of adjacent rounds never share a bank;
       the 8 tiles fill PSUM exactly.
  ACT: pure exp(x - C) stream on [128, 1024] tiles (a 16-step block is
       exp'd once and consumed over 16 rounds -> off the round path)
  DMA: host pre-packs the score image in bf16 (the matmul consumes bf16
       E-matrices regardless; exp(bf16(s)-C) vs bf16(exp(s-C)) is the same
       information loss) with every descriptor 2KB contiguous per partition
       (<512B descriptors pay a 2x RMW penalty).  32MB/core at ~330GB/s
       (~100us) leaves the kernel compute-paced: the fp32 wire measured
       295us with DMA 98% busy over the whole span.
"""

import os
import threading
import numpy as np
import ml_dtypes

L, B, T = 512, 64, 64
NCORES = 8
B_LOC = B // NCORES            # 8 sequences per core
NSEG = 8                       # segments per chain
NSTEP = 64                     # matrices per segment (incl. 1 identity pad)
NPAIR = 32                     # stream pairs per core: q = s*4 + a
NGP = NSEG // 2                # group-pairs (copy/PSUM granularity: 8 pairs)
W_LIST = [16, 16, 16, 16]      # steps per DMA/exp block (sum = NSTEP)
NBLK = len(W_LIST)
W_OFF = [sum(W_LIST[:k]) for k in range(NBLK)]
C_SHIFT = float(np.log(T) + 0.5)
START_TAG = 0
END_TAG = 1
NEG = -1e30                    # "minus infinity" for identity-pad off-diagonals
# bf16 rounding of the pad diagonal: the device applies exp(bf16(C) - C) != 1
# on each pad/masked step; the host subtracts this known constant exactly.
C_BF = float(np.asarray(C_SHIFT, dtype=ml_dtypes.bfloat16))

_nc_cache = [None]
_nc_lock = threading.Lock()
LAST_RESULTS = [None]          # test.py reads exec_time_ns from here


def _build_nc():
    import concourse.bacc as bacc
    import concourse.mybir as mybir
    import concourse.tile as tile

    dt = mybir.dt
    nc = bacc.Bacc("TRN2", target_bir_lowering=False, debug=False)

    # [pair, partition, (step, u)] bf16 -- 16KB contiguous per partition;
    # each block DMA slices W_k*128B >= 2KB per partition.
    img_d = nc.declare_dram_parameter(
        "img", [NPAIR, 128, NSTEP * T], dt.bfloat16, isOutput=False
    )
    eye_d = nc.declare_dram_parameter("eye2", [128, T], dt.float32, isOutput=False)
    out_d = nc.declare_dram_parameter(
        "m_out", [128, NSEG * 4 * T], dt.float32, isOutput=True
    )

    with tile.TileContext(nc) as tc:
        with (
            tc.tile_pool(name="raw", bufs=8) as raw_pool,
            tc.tile_pool(name="exp", bufs=2) as exp_pool,
            tc.tile_pool(name="state", bufs=1) as state_pool,
            tc.tile_pool(name="psum", bufs=1, space="PSUM") as psum_pool,
        ):
            eye_stage = state_pool.tile([128, T], dt.float32)
            eye_bf = state_pool.tile([128, T], dt.bfloat16)
            bias_c = state_pool.tile([128, 1], dt.float32)
            nc.gpsimd.memset(bias_c[:], -C_SHIFT)
            out_stage = state_pool.tile([128, NSEG * 4 * T], dt.float32)
            # accumulators per (group-pair gp in {0,1}, parity): group g =
            # 2*gp + gi at cols gi*256 + a*64; chain 2a+h at partitions h*64.
            acc = [
                [
                    state_pool.tile([128, 8 * T], dt.bfloat16, name=f"acc_gp{gp}p{p}")
                    for p in range(2)
                ]
                for gp in range(NGP)
            ]
            # PSUM: one [128, 512] tile (= exactly one 2KB bank) per
            # (group-pair, parity); parity-0 tiles first so the two parities
            # of a group-pair land in different banks (PE-W vs DVE-R on the
            # same bank serialize in hardware).
            psum_tiles = {}
            for p in range(2):
                for gp in range(NGP):
                    psum_tiles[(gp, p)] = psum_pool.tile(
                        [128, 8 * T], dt.float32, name=f"ps_gp{gp}p{p}"
                    )

            nc.sync.dma_start(eye_stage[:], eye_d[:])
            nc.vector.tensor_copy(eye_bf[:], eye_stage[:])
            for gp in range(NGP):
                for col in range(8):
                    nc.vector.tensor_copy(
                        acc[gp][0][:, col * T : (col + 1) * T], eye_bf[:]
                    )

            # ---- stream in all score blocks (SP queue drains as raw bufs
            # free up; order (k, q) matches consumption order).  One exp
            # instruction per (pair, block): batching two pairs per exp
            # measured slower on HW (207us vs 199us) -- coarser exp
            # granularity delays the first consumer rounds. ----
            raw_tiles = {}
            for k in range(NBLK):
                for q in range(NPAIR):
                    t_raw = raw_pool.tile([128, W_LIST[k] * T], dt.bfloat16, tag="raw")
                    nc.sync.dma_start(
                        t_raw[:],
                        img_d[q, :, W_OFF[k] * T : (W_OFF[k] + W_LIST[k]) * T],
                    )
                    raw_tiles[(q, k)] = t_raw

            # All PSUM->SBUF copies stay on DVE: offloading some to ScalarE
            # measured SLOWER on hardware (238-262us vs 234us) -- switching
            # the ACT function between Exp and Copy reloads the activation
            # table (~1.3us per switch) and ACT's PSUM port is slower.
            exp_tiles = {}

            def emit_exp(q, k):
                t = exp_pool.tile([128, W_LIST[k] * T], dt.bfloat16, tag=f"exp{q}")
                nc.scalar.activation(
                    t[:],
                    raw_tiles[(q, k)][:],
                    mybir.ActivationFunctionType.Exp,
                    bias=bias_c[:, 0:1],
                )
                exp_tiles[(q, k)] = t

            for q in range(NPAIR):
                emit_exp(q, 0)

            # ---- 128 lockstep rounds over 32 independent streams ----
            for r in range(NSTEP):
                kblk = max(k for k in range(NBLK) if W_OFF[k] <= r)
                w = r - W_OFF[kblk]
                rp = r % 2
                last = r == NSTEP - 1
                for gp in range(NGP):
                    ps = psum_tiles[(gp, rp)]
                    a_in = acc[gp][rp]
                    for gi in range(2):
                        g = 2 * gp + gi
                        for a in range(4):
                            q = g * 4 + a
                            et = exp_tiles[(q, kblk)]
                            lo = slice(0, T)
                            hi = slice(T, 2 * T)
                            cw = slice(w * T, (w + 1) * T)
                            ca = slice(gi * 4 * T + a * T, gi * 4 * T + (a + 1) * T)
                            nc.tensor.matmul(
                                ps[lo, ca], et[lo, cw], a_in[lo, ca],
                                start=True, stop=True,
                            )
                            nc.tensor.matmul(
                                ps[hi, ca], et[hi, cw], a_in[hi, ca],
                                start=True, stop=True,
                            )
                    # one [128,512] DVE copy per group-pair: bf16 for the
                    # next round, fp32 to the staging tile on the last one.
                    # (Offloading copies to ScalarE measured slower on HW
                    # every time it was tried: 201/238/262us vs 199us.)
                    if last:
                        dst = out_stage[:, gp * 8 * T : (gp + 1) * 8 * T]
                    else:
                        dst = acc[gp][(r + 1) % 2][:]
                    nc.vector.tensor_copy(dst, ps[:])
                # spread next block's 32 exps over this block's rounds
                if kblk < NBLK - 1:
                    wk = W_LIST[kblk]
                    for qn in range(w * NPAIR // wk, (w + 1) * NPAIR // wk):
                        emit_exp(qn, kblk + 1)

            nc.sync.dma_start(out_d[:], out_stage[:])
    nc.compile()
    return nc


def _get_nc():
    with _nc_lock:
        if _nc_cache[0] is None:
            _nc_cache[0] = _build_nc()
        return _nc_cache[0]


def _ensure_axon_hooks():
    """Provide antenv.axon_hooks (missing in this image) so that
    run_bass_kernel_spmd(trace=True) can register the NTFF profile hook."""
    import sys
    import types

    try:
        import antenv.axon_hooks  # noqa: F401
        return
    except ImportError:
        pass
    import antenv

    mod = types.ModuleType("antenv.axon_hooks")
    _hook = [None]
    mod.set_axon_ntff_profile_hook = lambda h: _hook.__setitem__(0, h)
    mod.get_axon_ntff_profile_hook = lambda: _hook[0]
    sys.modules["antenv.axon_hooks"] = mod
    antenv.axon_hooks = mod
    try:
        from trn_agent_boot.trn_boot import _ntff_profile_via_ctypes

        h = _ntff_profile_via_ctypes("/opt/axon/libaxon_pjrt.so")
        if h is not None:
            mod.set_axon_ntff_profile_hook(h)
    except Exception:
        pass


def _build_image(scores, mask, mask_all, c):
    """Per-core DMA image [NPAIR, 128, NSTEP*T] bf16.

    img[(s,a), (h,t), (j,u)] = padded[s*128 + j, (a,h), t, u]
    where padded[0] is the identity-pad matrix (diag +C, off-diag -1e30 so
    exp(x - C) ~= I) and padded[m] = scores[m] for m >= 1.
    """
    sh = scores[:, c * B_LOC : (c + 1) * B_LOC]  # (512, 8, 64, 64) view
    padded = np.empty((L, B_LOC, T, T), dtype=np.float32)
    padded[1:] = sh[1:]
    pad = np.full((T, T), NEG, dtype=np.float32)
    np.fill_diagonal(pad, C_SHIFT)
    padded[0] = pad
    if not mask_all:
        # a masked step must leave the partition unchanged: exp(x - C) ~= I
        mloc = mask[:, c * B_LOC : (c + 1) * B_LOC]
        ls, lb = np.nonzero(~mloc)
        for li, bi in zip(ls, lb):
            if li >= 1:
                padded[li, bi] = pad
    padded = padded.astype(ml_dtypes.bfloat16)
    # (s, a, h, t, j, u) <- (m=(s,j), c=(a,h), t, u)
    v = padded.reshape(NSEG, NSTEP, 4, 2, T, T)
    img = np.ascontiguousarray(v.transpose(0, 2, 3, 4, 1, 5)).reshape(
        NPAIR, 128, NSTEP * T
    )
    return img


def kernel(scores, target, mask, antor_score, aid, **_unused):
    from concourse.bass_utils import run_bass_kernel_spmd

    scores = np.asarray(scores, dtype=np.float32)
    target = np.asarray(target)
    mask = np.asarray(mask)
    antor_score = np.asarray(antor_score, dtype=np.float32)
    aid = int(np.asarray(aid))
    assert scores.shape == (L, B, T, T), scores.shape

    mask_all = bool(mask.all())

    # ---- host prep: initial vectors + per-core DMA images ----
    p0 = scores[0, :, START_TAG, :].astype(np.float64)          # (B, T)
    s0 = p0.max(axis=1)                                          # (B,)
    w0 = np.exp(p0 - s0[:, None])                                # (B, T) f64

    eye2 = np.tile(np.eye(T, dtype=np.float32), (2, 1))          # (128, 64)

    imgs = [None] * NCORES
    threads = [
        threading.Thread(
            target=lambda c=c: imgs.__setitem__(
                c, _build_image(scores, mask, mask_all, c)
            )
        )
        for c in range(NCORES)
    ]
    for t in threads:
        t.start()
    for t in threads:
        t.join()

    in_maps = [{"img": imgs[c], "eye2": eye2} for c in range(NCORES)]

    nc = _get_nc()
    do_trace = bool(int(os.environ.get("KERNEL_TRACE", "0")))
    if do_trace:
        _ensure_axon_hooks()
    try:
        res = run_bass_kernel_spmd(nc, in_maps, list(range(NCORES)), trace=do_trace)
    except Exception:
        if not do_trace:
            raise
        res = run_bass_kernel_spmd(nc, in_maps, list(range(NCORES)), trace=False)
    LAST_RESULTS[0] = res

    # ---- host combine (float64): Z_b = log(w[END]) + renorms + 511*C + s0 ----
    # m_out[(h,t'), gp*512 + gi*256 + a*64 + n] = M_{chain 2a+h, seg 2gp+gi}
    Z = 0.0
    for c in range(NCORES):
        out = np.asarray(res.results[c]["m_out"], dtype=np.float64)
        for bl in range(B_LOC):
            a, h = bl // 2, bl % 2
            b = c * B_LOC + bl
            w = w0[b].copy()
            logacc = 0.0
            for s in range(NSEG):
                col = s * 4 * T + a * T
                M = out[h * T : (h + 1) * T, col : col + T]
                w = M @ w
                mx = w.max()
                w /= mx
                logacc += np.log(mx)
            # each pad/masked step applied exp(C_BF - C) instead of exactly 1
            npad = 1 if mask_all else 1 + int((~mask[1:, b]).sum())
            Z += (
                np.log(w[END_TAG]) + logacc + s0[b]
                + (L - 1) * C_SHIFT - npad * (C_BF - C_SHIFT)
            )

    maskf = mask.astype(np.float64)
    tg = np.take_along_axis(
        scores.reshape(L, B, T * T), np.asarray(target, np.int64)[:, :, None], axis=2
    )[..., 0]
    tg_energy = float((tg * maskf).sum())

    a = antor_score.astype(np.float64)
    wsm = np.exp(a - a.max())
    wsm /= wsm.sum()
    loss = (Z - tg_energy) * wsm[aid] / B
    return np.float32(loss)
